# revision 15
# baseline (speedup 1.0000x reference)
"""Noisy top-k (k=2) router for Trainium2, data-parallel over 8 NeuronCores.

Math: for each row of noisy = logits + noise, the top-2 softmax weights are
    w1 = sigmoid(v1 - v2),  w2 = sigmoid(v2 - v1)   (v1 >= v2 top-2 values)
and sigmoid(2*x - (v1 + v2)) equals w1 at x = v1 and w2 at x = v2. So the
scattered output is
    out[x] = (x >= v2) * sigmoid(2*x - (v1 + v2))
which needs no indices at all. The (x >= v2) compare runs on the exact fp32
input values, so selection is bit-exact as long as no row has v2 == v3 ties
(holds for these inputs).

Engine split per tile (so DVE / ACT / GPSIMD / DMA all run near the memory
roofline in parallel):
  - DVE:    noisy = logits + noise; per-row-group Max8 (top-2); final
            out = mask * sig
  - GPSIMD: mask = (noisy >= v2) via broadcast compare
  - ACT:    sig = sigmoid(2*noisy - (v1+v2)) fused via scale/bias
This codegen allows only ONE sync-wait per instruction, so tiny same-engine
"observer" ops (d0/d1a/d1b) absorb extra cross-engine joins before the
two-input consumers.
"""

import numpy as np

import concourse.bass as bass
import concourse.mybir as mybir
from concourse.tile import TileContext
from concourse.tile_rust import add_dep_helper
from concourse.bass_utils import run_bass_kernel_spmd

B = 262144
E = 64
N_CORES = 8
B_CORE = B // N_CORES  # 32768 rows per core

P = 128  # SBUF partitions
TR = 32  # rows per partition per tile
ROWS_PER_TILE = P * TR  # 4096
NT = B_CORE // ROWS_PER_TILE  # 8 tiles per core

_CACHE = {}

# test.py introspection: BassKernelResults of the most recent run
LAST_RESULT = None


def _legalize_waits(nc: "bass.Bass") -> None:
    """This walrus codegen accepts at most ONE sync-wait per instruction
    (two on EventSemaphore). Tile's wait assigner can emit more; split the
    excess into standalone EventSemaphore waits placed immediately before
    the instruction on the same engine (identical semantics: the engine
    blocks there instead)."""
    n = 0
    for fnb in nc.m.functions[0].blocks:
        out = []
        for inst in fnb.instructions:
            si = inst.sync_info
            cap = 2 if isinstance(inst, mybir.InstEventSemaphore) else 1
            if si is not None and len(si.on_wait) > cap:
                waits = list(si.on_wait)
                extra, keep = waits[:-cap], waits[-cap:]
                for c in range(0, len(extra), 2):
                    n += 1
                    out.append(
                        mybir.InstEventSemaphore(
                            name=f"EVW-{n}",
                            engine=inst.engine,
                            sync_info=mybir.SyncInfo(
                                on_wait=extra[c : c + 2], on_update=[]
                            ),
                        )
                    )
                inst.sync_info = mybir.SyncInfo(
                    on_wait=keep, on_update=list(si.on_update)
                )
            out.append(inst)
        fnb.instructions = out


def _build_nc() -> bass.Bass:
    nc = bass.Bass()
    f32 = mybir.dt.float32

    lg = nc.dram_tensor("logits", [B_CORE, E], f32, kind="ExternalInput")
    nz = nc.dram_tensor("noise", [B_CORE, E], f32, kind="ExternalInput")
    out = nc.dram_tensor("out", [B_CORE, E], f32, kind="ExternalOutput")

    # row = t*(P*TR) + p*TR + r -> partition p holds TR contiguous rows
    lgv = lg[:].rearrange("(t p r) e -> t p r e", p=P, r=TR)
    nzv = nz[:].rearrange("(t p r) e -> t p r e", p=P, r=TR)
    outv = out[:].rearrange("(t p r) e -> t p r e", p=P, r=TR)

    with TileContext(nc) as tc:
        with (
            tc.tile_pool(name="work", bufs=4) as work_pool,
            tc.tile_pool(name="small", bufs=4) as small_pool,
        ):
            first_max8 = []
            for t in range(NT):
                # noisy = logits + noise, computed by the DMA engines:
                # plain load of logits, then an SWDGE accumulate-add load
                # of noise into the same tile.
                noisy = work_pool.tile([P, TR, E], f32, tag="noisy")
                lg_dma = nc.sync.dma_start(out=noisy, in_=lgv[t])
                if t >= 2:
                    # cap input-DMA run-ahead so early loads don't flood the
                    # DMA queues ahead of the first accumulates
                    add_dep_helper(lg_dma.ins, first_max8[t - 2].ins, sync=True)
                nc.gpsimd.dma_start(
                    out=noisy, in_=nzv[t], accum_op=mybir.AluOpType.add
                )

                # top-8 per row; we use slots 0 (v1) and 1 (v2)
                v8 = small_pool.tile([P, TR, 8], f32, tag="v8")
                for r in range(TR):
                    m8 = nc.vector.max(out=v8[:, r, :], in_=noisy[:, r, :])
                    if r == 0:
                        first_max8.append(m8)

                # negs = -(v1 + v2) = (v1 * -1) - v2
                negs = small_pool.tile([P, TR], f32, tag="negs")
                nc.vector.scalar_tensor_tensor(
                    out=negs,
                    in0=v8[:, :, 0],
                    scalar=-1.0,
                    in1=v8[:, :, 1],
                    op0=mybir.AluOpType.mult,
                    op1=mybir.AluOpType.subtract,
                )

                # sig = sigmoid(2*noisy - (v1+v2))
                sig = work_pool.tile([P, TR, E], f32, tag="sig")
                for r in range(TR):
                    nc.scalar.activation(
                        out=sig[:, r, :],
                        in_=noisy[:, r, :],
                        func=mybir.ActivationFunctionType.Sigmoid,
                        bias=negs[:, r : r + 1],
                        scale=2.0,
                    )

                # mask = (noisy >= v2), exact fp32 compare, full width via
                # 0-step broadcast of the per-row v2 column
                mask = work_pool.tile([P, TR, E], f32, tag="mask")
                nc.vector.tensor_tensor(
                    out=mask,
                    in0=noisy,
                    in1=v8[:, :, 1].to_broadcast([P, TR, E]),
                    op=mybir.AluOpType.is_ge,
                )

                # out = mask * sig
                ot = work_pool.tile([P, TR, E], f32, tag="ot")
                nc.vector.tensor_tensor(
                    out=ot, in0=mask, in1=sig, op=mybir.AluOpType.mult
                )

                # output via the Activation HWDGE: separate queue set, so
                # stores never sit behind the input-load descriptors
                nc.scalar.dma_start(out=outv[t], in_=ot)

    _legalize_waits(nc)
    return nc


def _get_nc() -> bass.Bass:
    if "nc" not in _CACHE:
        _CACHE["nc"] = _build_nc()
    return _CACHE["nc"]


def kernel(logits: np.ndarray, noise: np.ndarray) -> np.ndarray:
    global LAST_RESULT
    logits = np.ascontiguousarray(np.asarray(logits), dtype=np.float32)
    noise = np.ascontiguousarray(np.asarray(noise), dtype=np.float32)
    assert logits.shape == (B, E) and noise.shape == (B, E)

    lg_shards = np.split(logits, N_CORES, axis=0)
    nz_shards = np.split(noise, N_CORES, axis=0)
    in_maps = [
        {"logits": lg_shards[i], "noise": nz_shards[i]} for i in range(N_CORES)
    ]

    res = run_bass_kernel_spmd(_get_nc(), in_maps, core_ids=list(range(N_CORES)))
    LAST_RESULT = res
    return np.concatenate([r["out"] for r in res.results], axis=0)


# revision 18
# speedup vs baseline: 1.1336x; 1.1336x over previous
"""Noisy top-k (k=2) router for Trainium2, data-parallel over 8 NeuronCores.

Math: for each row of noisy = logits + noise, the top-2 softmax weights are
    w1 = sigmoid(v1 - v2),  w2 = sigmoid(v2 - v1)   (v1 >= v2 top-2 values)
and sigmoid(2*x - (v1 + v2)) equals w1 at x = v1 and w2 at x = v2. So the
scattered output is
    out[x] = (x >= v2) * sigmoid(2*x - (v1 + v2))
which needs no indices at all. The (x >= v2) compare runs on the exact fp32
input values, so selection is bit-exact as long as no row has v2 == v3 ties
(holds for these inputs).

Engine split per tile (so DVE / ACT / GPSIMD / DMA all run near the memory
roofline in parallel):
  - DVE:    noisy = logits + noise; per-row-group Max8 (top-2); final
            out = mask * sig
  - GPSIMD: mask = (noisy >= v2) via broadcast compare
  - ACT:    sig = sigmoid(2*noisy - (v1+v2)) fused via scale/bias
This codegen allows only ONE sync-wait per instruction, so tiny same-engine
"observer" ops (d0/d1a/d1b) absorb extra cross-engine joins before the
two-input consumers.
"""

import numpy as np

import concourse.bass as bass
import concourse.mybir as mybir
from concourse.tile import TileContext
from concourse.tile_rust import add_dep_helper
from concourse.bass_utils import run_bass_kernel_spmd

B = 262144
E = 64
N_CORES = 8
B_CORE = B // N_CORES  # 32768 rows per core

P = 128  # SBUF partitions
TR = 32  # rows per partition per tile
ROWS_PER_TILE = P * TR  # 4096
NT = B_CORE // ROWS_PER_TILE  # 8 tiles per core

_CACHE = {}

# test.py introspection: BassKernelResults of the most recent run
LAST_RESULT = None


def _legalize_waits(nc: "bass.Bass") -> None:
    """This walrus codegen accepts at most ONE sync-wait per instruction
    (two on EventSemaphore). Tile's wait assigner can emit more; split the
    excess into standalone EventSemaphore waits placed immediately before
    the instruction on the same engine (identical semantics: the engine
    blocks there instead)."""
    n = 0
    for fnb in nc.m.functions[0].blocks:
        out = []
        for inst in fnb.instructions:
            si = inst.sync_info
            cap = 2 if isinstance(inst, mybir.InstEventSemaphore) else 1
            if si is not None and len(si.on_wait) > cap:
                waits = list(si.on_wait)
                extra, keep = waits[:-cap], waits[-cap:]
                for c in range(0, len(extra), 2):
                    n += 1
                    out.append(
                        mybir.InstEventSemaphore(
                            name=f"EVW-{n}",
                            engine=inst.engine,
                            sync_info=mybir.SyncInfo(
                                on_wait=extra[c : c + 2], on_update=[]
                            ),
                        )
                    )
                inst.sync_info = mybir.SyncInfo(
                    on_wait=keep, on_update=list(si.on_update)
                )
            out.append(inst)
        fnb.instructions = out


def _build_nc() -> bass.Bass:
    nc = bass.Bass()
    f32 = mybir.dt.float32

    lg = nc.dram_tensor("logits", [B_CORE, E], f32, kind="ExternalInput")
    nz = nc.dram_tensor("noise", [B_CORE, E], f32, kind="ExternalInput")
    out = nc.dram_tensor("out", [B_CORE, E], f32, kind="ExternalOutput")

    # row = t*(P*TR) + p*TR + r -> partition p holds TR contiguous rows
    lgv = lg[:].rearrange("(t p r) e -> t p r e", p=P, r=TR)
    nzv = nz[:].rearrange("(t p r) e -> t p r e", p=P, r=TR)
    outv = out[:].rearrange("(t p r) e -> t p r e", p=P, r=TR)

    with TileContext(nc) as tc:
        with (
            tc.tile_pool(name="work", bufs=4) as work_pool,
            tc.tile_pool(name="small", bufs=4) as small_pool,
        ):
            N_DVE_ADD = 2  # leading tiles add on DVE: faster pipeline start
            for t in range(NT):
                # noisy = logits + noise. Leading tiles: two plain loads +
                # DVE add (low latency). Later tiles: SWDGE accumulate-add
                # so the add runs on the DMA engines' CCE instead of DVE.
                noisy = work_pool.tile([P, TR, E], f32, tag="noisy")
                if t < N_DVE_ADD:
                    lgt = work_pool.tile([P, TR, E], f32, tag="lgt", bufs=2)
                    nzt = work_pool.tile([P, TR, E], f32, tag="nzt", bufs=2)
                    nc.sync.dma_start(out=lgt, in_=lgv[t])
                    nc.sync.dma_start(out=nzt, in_=nzv[t])
                    nc.vector.tensor_tensor(
                        out=noisy, in0=lgt, in1=nzt, op=mybir.AluOpType.add
                    )
                else:
                    nc.sync.dma_start(out=noisy, in_=lgv[t])
                    nc.gpsimd.dma_start(
                        out=noisy, in_=nzv[t], accum_op=mybir.AluOpType.add
                    )

                # top-8 per row; we use slots 0 (v1) and 1 (v2)
                v8 = small_pool.tile([P, TR, 8], f32, tag="v8")
                for r in range(TR):
                    nc.vector.max(out=v8[:, r, :], in_=noisy[:, r, :])

                # negs = -(v1 + v2) = (v1 * -1) - v2
                negs = small_pool.tile([P, TR], f32, tag="negs")
                nc.vector.scalar_tensor_tensor(
                    out=negs,
                    in0=v8[:, :, 0],
                    scalar=-1.0,
                    in1=v8[:, :, 1],
                    op0=mybir.AluOpType.mult,
                    op1=mybir.AluOpType.subtract,
                )

                # sig = sigmoid(2*noisy - (v1+v2))
                sig = work_pool.tile([P, TR, E], f32, tag="sig")
                for r in range(TR):
                    nc.scalar.activation(
                        out=sig[:, r, :],
                        in_=noisy[:, r, :],
                        func=mybir.ActivationFunctionType.Sigmoid,
                        bias=negs[:, r : r + 1],
                        scale=2.0,
                    )

                # mask = (noisy >= v2), exact fp32 compare, full width via
                # 0-step broadcast of the per-row v2 column
                mask = work_pool.tile([P, TR, E], f32, tag="mask")
                nc.vector.tensor_tensor(
                    out=mask,
                    in0=noisy,
                    in1=v8[:, :, 1].to_broadcast([P, TR, E]),
                    op=mybir.AluOpType.is_ge,
                )

                # out = mask * sig
                ot = work_pool.tile([P, TR, E], f32, tag="ot")
                nc.vector.tensor_tensor(
                    out=ot, in0=mask, in1=sig, op=mybir.AluOpType.mult
                )

                nc.sync.dma_start(out=outv[t], in_=ot)

    _legalize_waits(nc)
    return nc


def _get_nc() -> bass.Bass:
    if "nc" not in _CACHE:
        _CACHE["nc"] = _build_nc()
    return _CACHE["nc"]


def kernel(logits: np.ndarray, noise: np.ndarray) -> np.ndarray:
    global LAST_RESULT
    logits = np.ascontiguousarray(np.asarray(logits), dtype=np.float32)
    noise = np.ascontiguousarray(np.asarray(noise), dtype=np.float32)
    assert logits.shape == (B, E) and noise.shape == (B, E)

    lg_shards = np.split(logits, N_CORES, axis=0)
    nz_shards = np.split(noise, N_CORES, axis=0)
    in_maps = [
        {"logits": lg_shards[i], "noise": nz_shards[i]} for i in range(N_CORES)
    ]

    res = run_bass_kernel_spmd(_get_nc(), in_maps, core_ids=list(range(N_CORES)))
    LAST_RESULT = res
    return np.concatenate([r["out"] for r in res.results], axis=0)


# revision 25
# speedup vs baseline: 1.1884x; 1.0484x over previous
"""Noisy top-k (k=2) router for Trainium2, data-parallel over 8 NeuronCores.

Math: for each row of noisy = logits + noise, the top-2 softmax weights are
    w1 = sigmoid(v1 - v2),  w2 = sigmoid(v2 - v1)   (v1 >= v2 top-2 values)
and sigmoid(2*x - (v1 + v2)) equals w1 at x = v1 and w2 at x = v2. So the
scattered output is
    out[x] = (x >= v2) * sigmoid(2*x - (v1 + v2))
which needs no indices at all. The (x >= v2) compare runs on the exact fp32
input values, so selection is bit-exact as long as no row has v2 == v3 ties
(holds for these inputs).

Engine split per tile (so DVE / ACT / GPSIMD / DMA all run near the memory
roofline in parallel):
  - DVE:    noisy = logits + noise; per-row-group Max8 (top-2); final
            out = mask * sig
  - GPSIMD: mask = (noisy >= v2) via broadcast compare
  - ACT:    sig = sigmoid(2*noisy - (v1+v2)) fused via scale/bias
This codegen allows only ONE sync-wait per instruction, so tiny same-engine
"observer" ops (d0/d1a/d1b) absorb extra cross-engine joins before the
two-input consumers.
"""

import numpy as np

import concourse.bass as bass
import concourse.mybir as mybir
from concourse.tile import TileContext
from concourse.tile_rust import add_dep_helper
from concourse.bass_utils import run_bass_kernel_spmd

B = 262144
E = 64
N_CORES = 8
B_CORE = B // N_CORES  # 32768 rows per core

P = 128  # SBUF partitions
NC_ = 4  # DMA super-chunks per core (16KB contiguous per partition each)
NS = 4  # compute subtiles per chunk
TRS = 16  # rows per partition per subtile
# rows per partition total = NC_*NS*TRS = 256; row = ((p*NC_+c)*NS+s)*TRS+r

_CACHE = {}

# test.py introspection: BassKernelResults of the most recent run
LAST_RESULT = None


def _legalize_waits(nc: "bass.Bass") -> None:
    """This walrus codegen accepts at most ONE sync-wait per instruction
    (two on EventSemaphore). Tile's wait assigner can emit more; split the
    excess into standalone EventSemaphore waits placed immediately before
    the instruction on the same engine (identical semantics: the engine
    blocks there instead)."""
    n = 0
    for fnb in nc.m.functions[0].blocks:
        out = []
        for inst in fnb.instructions:
            si = inst.sync_info
            cap = 2 if isinstance(inst, mybir.InstEventSemaphore) else 1
            if si is not None and len(si.on_wait) > cap:
                waits = list(si.on_wait)
                extra, keep = waits[:-cap], waits[-cap:]
                for c in range(0, len(extra), 2):
                    n += 1
                    out.append(
                        mybir.InstEventSemaphore(
                            name=f"EVW-{n}",
                            engine=inst.engine,
                            sync_info=mybir.SyncInfo(
                                on_wait=extra[c : c + 2], on_update=[]
                            ),
                        )
                    )
                inst.sync_info = mybir.SyncInfo(
                    on_wait=keep, on_update=list(si.on_update)
                )
            out.append(inst)
        fnb.instructions = out


def _build_nc() -> bass.Bass:
    nc = bass.Bass()
    f32 = mybir.dt.float32

    lg = nc.dram_tensor("logits", [B_CORE, E], f32, kind="ExternalInput")
    nz = nc.dram_tensor("noise", [B_CORE, E], f32, kind="ExternalInput")
    out = nc.dram_tensor("out", [B_CORE, E], f32, kind="ExternalOutput")

    # partition-major layout: partition p owns 256 contiguous DRAM rows,
    # split into NC_ chunks of NS*TRS rows -> 16KB contiguous per partition
    # per chunk, so every DMA descriptor is 16KB
    lgv = lg[:].rearrange("(p c s r) e -> c p s r e", p=P, c=NC_, s=NS)
    nzv = nz[:].rearrange("(p c s r) e -> c p s r e", p=P, c=NC_, s=NS)
    outv = out[:].rearrange("(p c s r) e -> c p s r e", p=P, c=NC_, s=NS)

    with TileContext(nc) as tc:
        with (
            tc.tile_pool(name="chunk", bufs=3) as chunk_pool,
            tc.tile_pool(name="sub", bufs=6) as sub_pool,
        ):
            N_DVE_ADD = 1  # leading chunks add on DVE: faster pipeline start
            for c in range(NC_):
                # noisy = logits + noise. Leading chunk: two plain loads +
                # DVE add per subtile (low latency). Later chunks: SWDGE
                # accumulate-add on the DMA engines' CCE instead of DVE.
                noisy = chunk_pool.tile([P, NS, TRS, E], f32, tag="noisy")
                if c < N_DVE_ADD:
                    lgt = chunk_pool.tile(
                        [P, NS, TRS, E], f32, tag="lgt", bufs=1
                    )
                    nzt = chunk_pool.tile(
                        [P, NS, TRS, E], f32, tag="nzt", bufs=1
                    )
                    nc.sync.dma_start(out=lgt, in_=lgv[c])
                    nc.sync.dma_start(out=nzt, in_=nzv[c])
                    for s in range(NS):
                        nc.vector.tensor_tensor(
                            out=noisy[:, s],
                            in0=lgt[:, s],
                            in1=nzt[:, s],
                            op=mybir.AluOpType.add,
                        )
                else:
                    nc.sync.dma_start(out=noisy, in_=lgv[c])
                    # 16KB accumulate descriptors fail at runtime; split the
                    # accumulate into proven 8KB halves
                    h = NS // 2
                    nc.gpsimd.dma_start(
                        out=noisy[:, :h],
                        in_=nzv[c][:, :h],
                        accum_op=mybir.AluOpType.add,
                    )
                    nc.gpsimd.dma_start(
                        out=noisy[:, h:],
                        in_=nzv[c][:, h:],
                        accum_op=mybir.AluOpType.add,
                    )

                ot = chunk_pool.tile([P, NS, TRS, E], f32, tag="ot", bufs=2)
                for s in range(NS):
                    # top-8 per row; we use slots 0 (v1) and 1 (v2)
                    v8 = sub_pool.tile([P, TRS, 8], f32, tag="v8")
                    for r in range(TRS):
                        nc.vector.max(out=v8[:, r, :], in_=noisy[:, s, r, :])

                    # negs = -(v1 + v2) = (v1 * -1) - v2
                    negs = sub_pool.tile([P, TRS], f32, tag="negs")
                    nc.vector.scalar_tensor_tensor(
                        out=negs,
                        in0=v8[:, :, 0],
                        scalar=-1.0,
                        in1=v8[:, :, 1],
                        op0=mybir.AluOpType.mult,
                        op1=mybir.AluOpType.subtract,
                    )

                    # sig = sigmoid(2*noisy - (v1+v2))
                    sig = sub_pool.tile([P, TRS, E], f32, tag="sig")
                    for r in range(TRS):
                        nc.scalar.activation(
                            out=sig[:, r, :],
                            in_=noisy[:, s, r, :],
                            func=mybir.ActivationFunctionType.Sigmoid,
                            bias=negs[:, r : r + 1],
                            scale=2.0,
                        )

                    # mask = (noisy >= v2), exact fp32 compare, full width
                    # via 0-step broadcast of the per-row v2 column
                    mask = sub_pool.tile([P, TRS, E], f32, tag="mask")
                    nc.vector.tensor_tensor(
                        out=mask,
                        in0=noisy[:, s],
                        in1=v8[:, :, 1].to_broadcast([P, TRS, E]),
                        op=mybir.AluOpType.is_ge,
                    )

                    # out = mask * sig
                    nc.vector.tensor_tensor(
                        out=ot[:, s], in0=mask, in1=sig, op=mybir.AluOpType.mult
                    )

                nc.sync.dma_start(out=outv[c], in_=ot)

    _legalize_waits(nc)
    return nc


def _get_nc() -> bass.Bass:
    if "nc" not in _CACHE:
        _CACHE["nc"] = _build_nc()
    return _CACHE["nc"]


def kernel(logits: np.ndarray, noise: np.ndarray) -> np.ndarray:
    global LAST_RESULT
    logits = np.ascontiguousarray(np.asarray(logits), dtype=np.float32)
    noise = np.ascontiguousarray(np.asarray(noise), dtype=np.float32)
    assert logits.shape == (B, E) and noise.shape == (B, E)

    lg_shards = np.split(logits, N_CORES, axis=0)
    nz_shards = np.split(noise, N_CORES, axis=0)
    in_maps = [
        {"logits": lg_shards[i], "noise": nz_shards[i]} for i in range(N_CORES)
    ]

    res = run_bass_kernel_spmd(_get_nc(), in_maps, core_ids=list(range(N_CORES)))
    LAST_RESULT = res
    return np.concatenate([r["out"] for r in res.results], axis=0)


# revision 27
# speedup vs baseline: 1.1924x; 1.0033x over previous
"""Noisy top-k (k=2) router for Trainium2, data-parallel over 8 NeuronCores.

Math: for each row of noisy = logits + noise, the top-2 softmax weights are
    w1 = sigmoid(v1 - v2),  w2 = sigmoid(v2 - v1)   (v1 >= v2 top-2 values)
and sigmoid(2*x - (v1 + v2)) equals w1 at x = v1 and w2 at x = v2. So the
scattered output is
    out[x] = (x >= v2) * sigmoid(2*x - (v1 + v2))
which needs no indices at all. The (x >= v2) compare runs on the exact fp32
input values, so selection is bit-exact as long as no row has v2 == v3 ties
(holds for these inputs).

Engine split per tile (so DVE / ACT / GPSIMD / DMA all run near the memory
roofline in parallel):
  - DVE:    noisy = logits + noise; per-row-group Max8 (top-2); final
            out = mask * sig
  - GPSIMD: mask = (noisy >= v2) via broadcast compare
  - ACT:    sig = sigmoid(2*noisy - (v1+v2)) fused via scale/bias
This codegen allows only ONE sync-wait per instruction, so tiny same-engine
"observer" ops (d0/d1a/d1b) absorb extra cross-engine joins before the
two-input consumers.
"""

import numpy as np

import concourse.bass as bass
import concourse.mybir as mybir
from concourse.tile import TileContext
from concourse.tile_rust import add_dep_helper
from concourse.bass_utils import run_bass_kernel_spmd

B = 262144
E = 64
N_CORES = 8
B_CORE = B // N_CORES  # 32768 rows per core

P = 128  # SBUF partitions
NC_ = 4  # DMA super-chunks per core (16KB contiguous per partition each)
NS = 4  # compute subtiles per chunk
TRS = 16  # rows per partition per subtile
# rows per partition total = NC_*NS*TRS = 256; row = ((p*NC_+c)*NS+s)*TRS+r

_CACHE = {}

# test.py introspection: BassKernelResults of the most recent run
LAST_RESULT = None


def _legalize_waits(nc: "bass.Bass") -> None:
    """This walrus codegen accepts at most ONE sync-wait per instruction
    (two on EventSemaphore). Tile's wait assigner can emit more; split the
    excess into standalone EventSemaphore waits placed immediately before
    the instruction on the same engine (identical semantics: the engine
    blocks there instead)."""
    n = 0
    for fnb in nc.m.functions[0].blocks:
        out = []
        for inst in fnb.instructions:
            si = inst.sync_info
            cap = 2 if isinstance(inst, mybir.InstEventSemaphore) else 1
            if si is not None and len(si.on_wait) > cap:
                waits = list(si.on_wait)
                extra, keep = waits[:-cap], waits[-cap:]
                for c in range(0, len(extra), 2):
                    n += 1
                    out.append(
                        mybir.InstEventSemaphore(
                            name=f"EVW-{n}",
                            engine=inst.engine,
                            sync_info=mybir.SyncInfo(
                                on_wait=extra[c : c + 2], on_update=[]
                            ),
                        )
                    )
                inst.sync_info = mybir.SyncInfo(
                    on_wait=keep, on_update=list(si.on_update)
                )
            out.append(inst)
        fnb.instructions = out


def _build_nc() -> bass.Bass:
    nc = bass.Bass()
    f32 = mybir.dt.float32

    lg = nc.dram_tensor("logits", [B_CORE, E], f32, kind="ExternalInput")
    nz = nc.dram_tensor("noise", [B_CORE, E], f32, kind="ExternalInput")
    out = nc.dram_tensor("out", [B_CORE, E], f32, kind="ExternalOutput")

    # partition-major layout: partition p owns 256 contiguous DRAM rows,
    # split into NC_ chunks of NS*TRS rows -> 16KB contiguous per partition
    # per chunk, so every DMA descriptor is 16KB
    lgv = lg[:].rearrange("(p c s r) e -> c p s r e", p=P, c=NC_, s=NS)
    nzv = nz[:].rearrange("(p c s r) e -> c p s r e", p=P, c=NC_, s=NS)
    outv = out[:].rearrange("(p c s r) e -> c p s r e", p=P, c=NC_, s=NS)

    with TileContext(nc) as tc:
        with (
            tc.tile_pool(name="chunk", bufs=3) as chunk_pool,
            tc.tile_pool(name="sub", bufs=6) as sub_pool,
        ):
            N_DVE_ADD = 1  # leading chunks add on DVE: faster pipeline start
            for c in range(NC_):
                # noisy = logits + noise. Leading chunk: two plain loads +
                # DVE add per subtile (low latency). Later chunks: SWDGE
                # accumulate-add on the DMA engines' CCE instead of DVE.
                noisy = chunk_pool.tile([P, NS, TRS, E], f32, tag="noisy")
                if c < N_DVE_ADD:
                    lgt = chunk_pool.tile(
                        [P, NS, TRS, E], f32, tag="lgt", bufs=1
                    )
                    nzt = chunk_pool.tile(
                        [P, NS, TRS, E], f32, tag="nzt", bufs=1
                    )
                    # per-subtile loads + adds: compute starts after the
                    # first 4KB arrives instead of the whole 16KB chunk
                    for s in range(NS):
                        nc.sync.dma_start(out=lgt[:, s], in_=lgv[c][:, s])
                        nc.sync.dma_start(out=nzt[:, s], in_=nzv[c][:, s])
                        nc.vector.tensor_tensor(
                            out=noisy[:, s],
                            in0=lgt[:, s],
                            in1=nzt[:, s],
                            op=mybir.AluOpType.add,
                        )
                else:
                    nc.sync.dma_start(out=noisy, in_=lgv[c])
                    # 16KB accumulate descriptors fail at runtime; split the
                    # accumulate into proven 8KB halves
                    h = NS // 2
                    nc.gpsimd.dma_start(
                        out=noisy[:, :h],
                        in_=nzv[c][:, :h],
                        accum_op=mybir.AluOpType.add,
                    )
                    nc.gpsimd.dma_start(
                        out=noisy[:, h:],
                        in_=nzv[c][:, h:],
                        accum_op=mybir.AluOpType.add,
                    )

                ot = chunk_pool.tile([P, NS, TRS, E], f32, tag="ot", bufs=2)
                for s in range(NS):
                    # top-8 per row; we use slots 0 (v1) and 1 (v2)
                    v8 = sub_pool.tile([P, TRS, 8], f32, tag="v8")
                    for r in range(TRS):
                        nc.vector.max(out=v8[:, r, :], in_=noisy[:, s, r, :])

                    # negs = -(v1 + v2) = (v1 * -1) - v2
                    negs = sub_pool.tile([P, TRS], f32, tag="negs")
                    nc.vector.scalar_tensor_tensor(
                        out=negs,
                        in0=v8[:, :, 0],
                        scalar=-1.0,
                        in1=v8[:, :, 1],
                        op0=mybir.AluOpType.mult,
                        op1=mybir.AluOpType.subtract,
                    )

                    # sig = sigmoid(2*noisy - (v1+v2))
                    sig = sub_pool.tile([P, TRS, E], f32, tag="sig")
                    for r in range(TRS):
                        nc.scalar.activation(
                            out=sig[:, r, :],
                            in_=noisy[:, s, r, :],
                            func=mybir.ActivationFunctionType.Sigmoid,
                            bias=negs[:, r : r + 1],
                            scale=2.0,
                        )

                    # mask = (noisy >= v2), exact fp32 compare, full width
                    # via 0-step broadcast of the per-row v2 column
                    mask = sub_pool.tile([P, TRS, E], f32, tag="mask")
                    nc.vector.tensor_tensor(
                        out=mask,
                        in0=noisy[:, s],
                        in1=v8[:, :, 1].to_broadcast([P, TRS, E]),
                        op=mybir.AluOpType.is_ge,
                    )

                    # out = mask * sig
                    nc.vector.tensor_tensor(
                        out=ot[:, s], in0=mask, in1=sig, op=mybir.AluOpType.mult
                    )

                    if s == NS // 2 - 1:
                        nc.sync.dma_start(
                            out=outv[c][:, : NS // 2], in_=ot[:, : NS // 2]
                        )
                nc.sync.dma_start(
                    out=outv[c][:, NS // 2 :], in_=ot[:, NS // 2 :]
                )

    _legalize_waits(nc)
    return nc


def _get_nc() -> bass.Bass:
    if "nc" not in _CACHE:
        _CACHE["nc"] = _build_nc()
    return _CACHE["nc"]


def kernel(logits: np.ndarray, noise: np.ndarray) -> np.ndarray:
    global LAST_RESULT
    logits = np.ascontiguousarray(np.asarray(logits), dtype=np.float32)
    noise = np.ascontiguousarray(np.asarray(noise), dtype=np.float32)
    assert logits.shape == (B, E) and noise.shape == (B, E)

    lg_shards = np.split(logits, N_CORES, axis=0)
    nz_shards = np.split(noise, N_CORES, axis=0)
    in_maps = [
        {"logits": lg_shards[i], "noise": nz_shards[i]} for i in range(N_CORES)
    ]

    res = run_bass_kernel_spmd(_get_nc(), in_maps, core_ids=list(range(N_CORES)))
    LAST_RESULT = res
    return np.concatenate([r["out"] for r in res.results], axis=0)
